# revision 1
# baseline (speedup 1.0000x reference)
"""DLRM forward (bottom MLP + embedding gather + dot interaction + top MLP)
on 8 Trainium2 NeuronCores via Bass/Tile.

Strategy: replicate the embedding table (1 GB) on every core's HBM and
data-parallel shard the batch 8 ways (4096 rows/core).  No collectives.
Per core:
  - bottom MLP computed in transposed layout (features on partitions,
    batch on the free dim)
  - embedding rows fetched with indirect-DMA gathers (128 rows/instr),
    PE-transposed into [embed, batch] layout
  - the 27x27 dot-interaction is one small self-loading matmul per sample
    (lhsT = rhs = strided column view of the transposed embeddings)
  - the upper-triangle extraction is folded into the first top-MLP layer:
    flat @ W.T == sum_{n,m} W729[f,n,m] * inter[n,m] with W729 the
    symmetrized (0.5 off-diag) expansion of tw0[:, 128:], so the top MLP
    contracts the full 27x27 Gram directly - no scatter/gather needed.
"""

import numpy as np
from contextlib import ExitStack

import concourse.bass as bass
import concourse.tile as tile
from concourse import bacc, mybir
from concourse.bass import IndirectOffsetOnAxis
from concourse.masks import make_identity

F32 = mybir.dt.float32
I32 = mybir.dt.int32
AF = mybir.ActivationFunctionType

VOCAB = 2_000_000
BATCH = 32768
ND = 13          # dense features
NS = 26          # sparse features
NF = NS + 1      # interaction features (h + embeddings)
EMB = 128
KD = 16          # dense K padded to 16 partitions
N_CORES = 8

BANKB = 16       # quads per PSUM bank (16 quads * 27 = 432 <= 512 fp32)
CHUNK_BANKS = 16 # banks per S-chunk for the top MLP
CH_QUADS = CHUNK_BANKS * BANKB   # quad-columns per chunk (per strip)
CH_COLS = CH_QUADS


def _emit(ctx, tc, t, B, V):
    """Emit the per-core program. t = dict of dram tensor APs."""
    nc = tc.nc
    T = B // 128            # b-tiles
    assert B % 128 == 0 and B % BANKB == 0
    NBANK = B // BANKB
    MLPC = min(512, B)      # bottom-MLP batch chunk
    NCH = B // MLPC

    const = ctx.enter_context(tc.tile_pool(name="const", bufs=1))
    mlp_sb = ctx.enter_context(tc.tile_pool(name="mlp_sb", bufs=2))
    ps_pool = ctx.enter_context(tc.tile_pool(name="ps", bufs=2, space="PSUM"))
    g_pool = ctx.enter_context(tc.tile_pool(name="g", bufs=12))
    gt_pool = ctx.enter_context(tc.tile_pool(name="gt", bufs=3))
    tp_pool = ctx.enter_context(tc.tile_pool(name="tp", bufs=3, space="PSUM"))
    it_pool = ctx.enter_context(tc.tile_pool(name="it", bufs=3, space="PSUM"))
    s_pool = ctx.enter_context(tc.tile_pool(name="s", bufs=2))
    z_pool = ctx.enter_context(tc.tile_pool(name="z", bufs=2))

    # ---- load constants/weights into SBUF ----
    def load(name, shape, dtype=F32):
        sb = const.tile(shape, dtype, tag=name)
        nc.sync.dma_start(sb[:], t[name])
        return sb

    bw0T = load("bw0T", [KD, 512])
    bw1T = load("bw1T", [128, 4 * 256])
    bw2T = load("bw2T", [128, 2 * 128])
    bb0 = load("bb0", [128, 4])
    bb1 = load("bb1", [128, 2])
    bb2 = load("bb2", [128, 1])
    w729s = load("w729s", [128, NF * 256])
    twhT = load("twhT", [128, 256])
    tw1T = load("tw1T", [128, 2 * 256])
    tw2T = load("tw2T", [128, 2 * 256])
    tw3T = load("tw3T", [128, 2 * 256])
    tw4T = load("tw4T", [128, 2])
    tb0 = load("tb0", [128, 2])
    tb1 = load("tb1", [128, 2])
    tb2 = load("tb2", [128, 2])
    tb3 = load("tb3", [128, 2])
    tb4 = load("tb4", [1, 1])

    idx_sb = const.tile([128, T * NS], I32)
    for tt in range(T):
        nc.sync.dma_start(
            idx_sb[:, tt * NS:(tt + 1) * NS],
            t["idx"][tt * 128:(tt + 1) * 128, :],
        )

    ident = const.tile([128, 128], F32)
    make_identity(nc, ident[:])

    hT = const.tile([128, B], F32)       # bottom-MLP output, transposed

    # ---- bottom MLP (transposed: out[f, b]) ----
    for ch in range(NCH):
        bs = slice(ch * MLPC, (ch + 1) * MLPC)
        xd = mlp_sb.tile([KD, MLPC], F32, tag="xd")
        nc.sync.dma_start(xd[:], t["xdT"][:, bs])
        h0 = mlp_sb.tile([128, 4 * MLPC], F32, tag="h0")
        for m in range(4):
            psf = ps_pool.tile([128, 512], F32, tag="ps")
            ps = psf[:, 0:MLPC]
            nc.tensor.matmul(ps, bw0T[:, m * 128:(m + 1) * 128], xd[:],
                             start=True, stop=True)
            nc.scalar.activation(h0[:, m * MLPC:(m + 1) * MLPC], ps,
                                 AF.Relu, bias=bb0[:, m:m + 1])
        h1 = mlp_sb.tile([128, 2 * MLPC], F32, tag="h1")
        for m in range(2):
            psf = ps_pool.tile([128, 512], F32, tag="ps")
            ps = psf[:, 0:MLPC]
            for k in range(4):
                nc.tensor.matmul(ps, bw1T[:, k * 256 + m * 128: k * 256 + (m + 1) * 128],
                                 h0[:, k * MLPC:(k + 1) * MLPC],
                                 start=(k == 0), stop=(k == 3))
            nc.scalar.activation(h1[:, m * MLPC:(m + 1) * MLPC], ps,
                                 AF.Relu, bias=bb1[:, m:m + 1])
        psf = ps_pool.tile([128, 512], F32, tag="ps")
        ps = psf[:, 0:MLPC]
        for k in range(2):
            nc.tensor.matmul(ps, bw2T[:, k * 128:(k + 1) * 128],
                             h1[:, k * MLPC:(k + 1) * MLPC],
                             start=(k == 0), stop=(k == 1))
        nc.scalar.activation(hT[:, bs], ps, AF.Relu, bias=bb2[:, 0:1])

    # ---- top MLP on one S-chunk (ncols batch columns from cstart) ----
    hT_ap = hT[:]

    def top_chunk(S, nb, cstart):
        ncols = nb * BANKB
        assert cstart + ncols <= B
        z1 = z_pool.tile([128, 2 * CH_COLS], F32, tag="z1")
        s_ap = S[:]
        pstride = s_ap.ap[0][0]
        for half in range(2):
            psf = ps_pool.tile([128, 512], F32, tag="ps")
            ps = psf[:, 0:ncols]
            nc.tensor.matmul(ps, twhT[:, half * 128:(half + 1) * 128],
                             hT_ap[:, cstart:cstart + ncols], start=True, stop=False)
            for m in range(NF):
                rhs = bass.AP(s_ap.tensor, s_ap.offset + m,
                              [[pstride, NF], [BANKB * NF, nb], [NF, BANKB]])
                lhsT = w729s[0:NF, m * 256 + half * 128: m * 256 + (half + 1) * 128]
                nc.tensor.matmul(ps, lhsT, rhs, start=False, stop=(m == NF - 1))
            nc.scalar.activation(z1[:, half * CH_COLS: half * CH_COLS + ncols], ps,
                                 AF.Relu, bias=tb0[:, half:half + 1])
        zp = z1
        for wT, bb in ((tw1T, tb1), (tw2T, tb2), (tw3T, tb3)):
            zn = z_pool.tile([128, 2 * CH_COLS], F32, tag="zn")
            for half in range(2):
                psf = ps_pool.tile([128, 512], F32, tag="ps")
                ps = psf[:, 0:ncols]
                for k in range(2):
                    nc.tensor.matmul(ps, wT[:, k * 256 + half * 128: k * 256 + (half + 1) * 128],
                                     zp[:, k * CH_COLS: k * CH_COLS + ncols],
                                     start=(k == 0), stop=(k == 1))
                nc.scalar.activation(zn[:, half * CH_COLS: half * CH_COLS + ncols], ps,
                                     AF.Relu, bias=bb[:, half:half + 1])
            zp = zn
        psf = ps_pool.tile([1, 512], F32, tag="ps")
        ps = psf[:, 0:ncols]
        for k in range(2):
            nc.tensor.matmul(ps, tw4T[:, k:k + 1],
                             zp[:, k * CH_COLS: k * CH_COLS + ncols],
                             start=(k == 0), stop=(k == 1))
        o5 = z_pool.tile([1, CH_COLS], F32, tag="o5")
        nc.scalar.activation(o5[0:1, 0:ncols], ps[0:1, 0:ncols],
                             AF.Identity, bias=tb4[0:1, 0:1])
        nc.sync.dma_start(t["out"][cstart:cstart + ncols, :], o5[0:1, 0:ncols])

    # ---- gather + transpose + per-sample interaction matmuls ----
    inter = None
    S = None
    chunk_nb = 0
    chunk_start = 0
    for tt in range(T):
        gt = gt_pool.tile([128, NF * 128], F32, tag="gt")
        nc.vector.tensor_copy(gt[:, 0:128], hT[:, tt * 128:(tt + 1) * 128])
        for n in range(NS):
            g = g_pool.tile([128, 128], F32, tag="g")
            nc.gpsimd.indirect_dma_start(
                out=g[:], out_offset=None, in_=t["table"],
                in_offset=IndirectOffsetOnAxis(
                    ap=idx_sb[:, tt * NS + n: tt * NS + n + 1], axis=0),
            )
            tp = tp_pool.tile([128, 128], F32, tag="tp")
            nc.tensor.transpose(tp[:], g[:], ident[:])
            dst = gt[:, (n + 1) * 128:(n + 2) * 128]
            if n % 4 == 3:
                nc.scalar.activation(dst, tp[:], AF.Copy)
            else:
                nc.vector.tensor_copy(dst, tp[:])
        gt_ap = gt[:]
        for s in range(128):
            b = tt * 128 + s
            bank = b // BANKB
            c = b % BANKB
            if c == 0:
                inter = it_pool.tile([NF, 512], F32, tag="it")
            col = bass.AP(gt_ap.tensor, gt_ap.offset + s,
                          [[gt_ap.ap[0][0], 128], [128, NF]])
            nc.tensor.matmul(inter[0:NF, c * NF:(c + 1) * NF], col, col,
                             start=(c == 0), stop=(c == BANKB - 1),
                             skip_group_check=True)
            if c == BANKB - 1:
                if bank % CHUNK_BANKS == 0:
                    S = s_pool.tile([NF, CHUNK_BANKS * BANKB * NF], F32, tag="s")
                    chunk_start = bank * BANKB
                    chunk_nb = 0
                nc.vector.tensor_copy(
                    S[:, chunk_nb * BANKB * NF:(chunk_nb + 1) * BANKB * NF],
                    inter[0:NF, 0:BANKB * NF])
                chunk_nb += 1
                if chunk_nb == CHUNK_BANKS or bank == NBANK - 1:
                    top_chunk(S, chunk_nb, chunk_start)


def build_program(B=BATCH // N_CORES, V=VOCAB):
    nc = bacc.Bacc("TRN2", target_bir_lowering=False, debug=False)
    t = {}

    def din(name, shape, dtype=F32):
        t[name] = nc.dram_tensor(name, shape, dtype, kind="ExternalInput").ap()

    din("xdT", [KD, B])
    din("idx", [B, NS], I32)
    din("table", [V, EMB])
    din("bw0T", [KD, 512])
    din("bw1T", [128, 4 * 256])
    din("bw2T", [128, 2 * 128])
    din("bb0", [128, 4])
    din("bb1", [128, 2])
    din("bb2", [128, 1])
    din("w729s", [128, NF * 256])
    din("twhT", [128, 256])
    din("tw1T", [128, 2 * 256])
    din("tw2T", [128, 2 * 256])
    din("tw3T", [128, 2 * 256])
    din("tw4T", [128, 2])
    din("tb0", [128, 2])
    din("tb1", [128, 2])
    din("tb2", [128, 2])
    din("tb3", [128, 2])
    din("tb4", [1, 1])
    t["out"] = nc.dram_tensor("out", [B, 1], F32, kind="ExternalOutput").ap()

    with tile.TileContext(nc) as tc:
        with ExitStack() as ctx:
            _emit(ctx, tc, t, B, V)
    nc.compile()
    return nc


def _chunked(wT, kdim):
    """[K, M] -> [128, (K//128)*M] with k-chunk k at cols [k*M, (k+1)*M)."""
    K, M = wT.shape
    assert K == kdim and K % 128 == 0
    return np.ascontiguousarray(
        np.concatenate([wT[k * 128:(k + 1) * 128] for k in range(K // 128)], axis=1),
        dtype=np.float32)


def host_prep(inputs, V=VOCAB):
    """Build the common (weight) input map + full xdT / idx arrays."""
    x = np.asarray(inputs["x"], np.float32)
    Bfull = x.shape[0]
    dense = x[:, :ND]
    idx = (x[:, ND:].astype(np.int64) % V).astype(np.int32)
    table = np.ascontiguousarray(
        np.concatenate([np.asarray(inputs[f"emb{i}"], np.float32) for i in range(4)], axis=0))
    assert table.shape[0] == V

    xdT = np.zeros((KD, Bfull), np.float32)
    xdT[:ND] = dense.T

    bw0 = np.asarray(inputs["bw0"], np.float32)
    bw0T = np.zeros((KD, 512), np.float32)
    bw0T[:ND] = bw0.T

    tw0 = np.asarray(inputs["tw0"], np.float32)
    iu = np.triu_indices(NF)
    Wf = np.zeros((256, NF, NF), np.float32)
    Wf[:, iu[0], iu[1]] = tw0[:, EMB:]
    Wf = 0.5 * (Wf + Wf.transpose(0, 2, 1))
    w729n = Wf.transpose(1, 2, 0).reshape(NF, NF * 256)
    w729s = np.zeros((128, NF * 256), np.float32)
    for j in range(4):
        w729s[32 * j:32 * j + NF] = w729n

    def pbias(b, m):
        b = np.asarray(b, np.float32)
        return np.ascontiguousarray(b.reshape(m, 128).T)

    common = {
        "table": table,
        "bw0T": bw0T,
        "bw1T": _chunked(np.asarray(inputs["bw1"], np.float32).T, 512),
        "bw2T": _chunked(np.asarray(inputs["bw2"], np.float32).T, 256),
        "bb0": pbias(inputs["bb0"], 4),
        "bb1": pbias(inputs["bb1"], 2),
        "bb2": pbias(inputs["bb2"], 1),
        "w729s": w729s,
        "twhT": np.ascontiguousarray(tw0[:, :EMB].T),
        "tw1T": _chunked(np.asarray(inputs["tw1"], np.float32).T, 256),
        "tw2T": _chunked(np.asarray(inputs["tw2"], np.float32).T, 256),
        "tw3T": _chunked(np.asarray(inputs["tw3"], np.float32).T, 256),
        "tw4T": _chunked(np.asarray(inputs["tw4"], np.float32).T, 256),
        "tb0": pbias(inputs["tb0"], 2),
        "tb1": pbias(inputs["tb1"], 2),
        "tb2": pbias(inputs["tb2"], 2),
        "tb3": pbias(inputs["tb3"], 2),
        "tb4": np.asarray(inputs["tb4"], np.float32).reshape(1, 1),
    }
    return common, xdT, idx


_CACHE = {}


def _build_exec(nc, n_cores):
    """Mirror bass2jax.run_bass_via_pjrt but return a reusable jitted fn."""
    import jax
    from jax.sharding import Mesh, PartitionSpec, NamedSharding
    from jax.experimental.shard_map import shard_map
    import concourse.mybir as mybir
    from concourse import bass2jax as B2J

    B2J.install_neuronx_cc_hook()
    pname = nc.partition_id_tensor.name if nc.partition_id_tensor else None
    in_names, out_names, out_avals, zero_outs = [], [], [], []
    for alloc in nc.m.functions[0].allocations:
        if not isinstance(alloc, mybir.MemoryLocationSet):
            continue
        name = alloc.memorylocations[0].name
        if alloc.kind == "ExternalInput":
            if name != pname:
                in_names.append(name)
        elif alloc.kind == "ExternalOutput":
            shape = tuple(alloc.tensor_shape)
            dtype = mybir.dt.np(alloc.dtype)
            out_names.append(name)
            out_avals.append(jax.core.ShapedArray(shape, dtype))
            zero_outs.append(np.zeros(shape, dtype))
    n_params = len(in_names)
    all_names = in_names + out_names
    if pname is not None:
        all_names = all_names + [pname]
    donate = tuple(range(n_params, n_params + len(out_names)))

    def _body(*args):
        operands = list(args)
        if pname is not None:
            operands.append(B2J.partition_id_tensor())
        outs = B2J._bass_exec_p.bind(
            *operands, out_avals=tuple(out_avals), in_names=tuple(all_names),
            out_names=tuple(out_names), lowering_input_output_aliases=(),
            sim_require_finite=True, sim_require_nnan=True, nc=nc)
        return tuple(outs)

    devices = jax.devices()[:n_cores]
    mesh = Mesh(np.asarray(devices), ("core",))
    nsh = NamedSharding(mesh, PartitionSpec("core"))
    in_specs = (PartitionSpec("core"),) * (n_params + len(out_names))
    out_specs = (PartitionSpec("core"),) * len(out_names)
    sharded = jax.jit(
        shard_map(_body, mesh=mesh, in_specs=in_specs, out_specs=out_specs,
                  check_rep=False),
        donate_argnums=donate, keep_unused=True)
    return dict(fn=sharded, in_names=in_names, out_names=out_names,
                zero_outs=zero_outs, sharding=nsh, n_cores=n_cores)


def _build_loop_exec(nc, n_cores, K):
    """K chained executions in one XLA program (for timing): iteration i+1's
    tb4 input gets + 0*out_i[0,0], forcing sequential device execution."""
    import jax
    from jax.sharding import Mesh, PartitionSpec
    from jax.experimental.shard_map import shard_map
    import concourse.mybir as mybir
    from concourse import bass2jax as B2J

    B2J.install_neuronx_cc_hook()
    pname = nc.partition_id_tensor.name if nc.partition_id_tensor else None
    in_names, out_names, out_avals, zero_outs = [], [], [], []
    for alloc in nc.m.functions[0].allocations:
        if not isinstance(alloc, mybir.MemoryLocationSet):
            continue
        name = alloc.memorylocations[0].name
        if alloc.kind == "ExternalInput":
            if name != pname:
                in_names.append(name)
        elif alloc.kind == "ExternalOutput":
            out_names.append(name)
            out_avals.append(jax.core.ShapedArray(
                tuple(alloc.tensor_shape), mybir.dt.np(alloc.dtype)))
            zero_outs.append(np.zeros(tuple(alloc.tensor_shape),
                                      mybir.dt.np(alloc.dtype)))
    n_params = len(in_names)
    all_names = in_names + out_names
    if pname is not None:
        all_names = all_names + [pname]
    tb4_i = in_names.index("tb4")

    def _body(*args):
        params = list(args[:n_params])
        zeros = list(args[n_params:])
        outs = None
        for _ in range(K):
            ops = list(params)
            if outs is not None:
                eps = outs[0][0, 0] * 0.0
                ops[tb4_i] = ops[tb4_i] + eps
            if pname is not None:
                ops = ops + zeros + [B2J.partition_id_tensor()]
            else:
                ops = ops + zeros
            outs = B2J._bass_exec_p.bind(
                *ops, out_avals=tuple(out_avals), in_names=tuple(all_names),
                out_names=tuple(out_names), lowering_input_output_aliases=(),
                sim_require_finite=True, sim_require_nnan=True, nc=nc)
        return tuple(outs)

    devices = jax.devices()[:n_cores]
    mesh = Mesh(np.asarray(devices), ("core",))
    in_specs = (PartitionSpec("core"),) * (n_params + len(out_names))
    out_specs = (PartitionSpec("core"),) * len(out_names)
    return jax.jit(shard_map(_body, mesh=mesh, in_specs=in_specs,
                             out_specs=out_specs, check_rep=False),
                   keep_unused=True)


def time_loop(K=8, reps=6):
    """Time K-chained vs 1-chained executions; return per-exec ns estimate."""
    import time as _time
    import jax
    ex = _CACHE["exec"]
    nc = _CACHE["nc"]
    zeros = [jax.device_put(
        np.zeros((ex["n_cores"] * z.shape[0], *z.shape[1:]), z.dtype),
        ex["sharding"]) for z in ex["zero_outs"]]

    def bench(fn):
        ts = []
        jax.block_until_ready(fn(*_CACHE["dev_in"], *zeros))  # warm
        for _ in range(reps):
            t0 = _time.perf_counter()
            jax.block_until_ready(fn(*_CACHE["dev_in"], *zeros))
            ts.append(_time.perf_counter() - t0)
        return np.array(ts)

    fK = _CACHE.get(("loop", K)) or _build_loop_exec(nc, ex["n_cores"], K)
    _CACHE[("loop", K)] = fK
    f1 = _CACHE.get(("loop", 1)) or _build_loop_exec(nc, ex["n_cores"], 1)
    _CACHE[("loop", 1)] = f1
    tK = bench(fK)
    t1 = bench(f1)
    per_exec = (np.min(tK) - np.min(t1)) / (K - 1)
    return per_exec * 1e9, tK, t1


def _stage_inputs(inputs):
    """host_prep + concat + device_put; cached on input identity."""
    import jax
    key = id(inputs.get("x"))
    if _CACHE.get("staged_key") == key:
        return
    B = BATCH // N_CORES
    common, xdT, idx = host_prep(inputs, VOCAB)
    ex = _CACHE["exec"]
    dev_in = []
    for name in ex["in_names"]:
        if name == "xdT":
            arr = np.concatenate(
                [xdT[:, c * B:(c + 1) * B] for c in range(N_CORES)], axis=0)
        elif name == "idx":
            arr = np.ascontiguousarray(idx)  # [BATCH, NS] == concat of shards
        else:
            v = common[name]
            arr = np.concatenate([v] * N_CORES, axis=0)
        dev_in.append(jax.device_put(arr, ex["sharding"]))
    jax.block_until_ready(dev_in)
    _CACHE["dev_in"] = dev_in
    _CACHE["staged_key"] = key


def _run_staged():
    import jax
    ex = _CACHE["exec"]
    zeros = [jax.device_put(
        np.zeros((ex["n_cores"] * z.shape[0], *z.shape[1:]), z.dtype),
        ex["sharding"]) for z in ex["zero_outs"]]
    out = ex["fn"](*_CACHE["dev_in"], *zeros)
    jax.block_until_ready(out)
    return out


def kernel(**inputs):
    from concourse.bass_utils import run_bass_kernel_spmd

    B = BATCH // N_CORES
    if "nc" not in _CACHE:
        _CACHE["nc"] = build_program(B, VOCAB)
    nc = _CACHE["nc"]
    common, xdT, idx = host_prep(inputs, VOCAB)
    in_maps = []
    for c in range(N_CORES):
        m = dict(common)
        m["xdT"] = np.ascontiguousarray(xdT[:, c * B:(c + 1) * B])
        m["idx"] = np.ascontiguousarray(idx[c * B:(c + 1) * B])
        in_maps.append(m)
    res = run_bass_kernel_spmd(nc, in_maps, core_ids=list(range(N_CORES)))
    _CACHE["last_results"] = res
    return np.concatenate([r["out"] for r in res.results], axis=0)

